# revision 1
# baseline (speedup 1.0000x reference)
"""Trainium2 Bass kernel v2 for temporal attention (nn_Attention_4423816315129).

Transpose-free design: q/k are computed directly in head-dim-on-partitions
layout (w stationary, xT moving), so sim = S^T comes out of one matmul per
(head, 128-token block) with no PE transposes.  Softmax runs in the
j-on-partitions layout: partition sums via gpsimd.partition_all_reduce,
masking+pos_bias via multiply-by-exp(pb) table (masked entries become 0).
AV emits Y^T directly (col-sliced PSUM writes), feeding the out-projection
without any transposes.  Per 512-token tile: 128 matmuls, ~17 vector ops.

Sharding: hw axis split across 8 cores, pure data parallel.
"""

import numpy as np
import ml_dtypes

import concourse.bass as bass
from concourse import bacc, bass_isa
import concourse.mybir as mybir
import concourse.tile as tile
from concourse.bass import ts
from concourse.bass_utils import run_bass_kernel_spmd

HEADS = 8
DIM_HEAD = 64
B = 2
HW = 1024
N = 32
DIM = 512
N_CORES = 8
HW_SHARD = HW // N_CORES            # 128
TOK = B * HW_SHARD * N              # 8192 tokens per core
TILE_T = 512                        # tokens per tile (16 groups of 32)
N_TILES = TOK // TILE_T             # 16
BF16 = mybir.dt.bfloat16
F32 = mybir.dt.float32

# feature flags (fallbacks if a primitive is unsupported)
SIM_BASE64 = True      # sim operands at partition base 64 for odd heads
YT_COLSLICE = True     # AV writes Y^T into col-sliced PSUM (base 0/64)
GPS_REDUCE = True      # gpsimd.partition_all_reduce for softmax sums

Exp = mybir.ActivationFunctionType.Exp
Mult = mybir.AluOpType.mult


def build_nc(mask_flags, repeat=1, stage="full", n_tiles=N_TILES):
    nc = bacc.Bacc("TRN2", target_bir_lowering=False)

    x_d = nc.dram_tensor("x", [N_TILES * 128, 4 * TILE_T], BF16, kind="ExternalInput")
    wq_d = nc.dram_tensor("wq", [DIM, DIM], BF16, kind="ExternalInput")
    wkv_d = nc.dram_tensor("wkv", [DIM, 2 * DIM], BF16, kind="ExternalInput")
    wo_d = nc.dram_tensor("wo", [DIM, DIM], BF16, kind="ExternalInput")
    cos_d = nc.dram_tensor("cosb", [128, TILE_T], BF16, kind="ExternalInput")
    sin_d = nc.dram_tensor("sinb", [128, TILE_T], BF16, kind="ExternalInput")
    pb_d = nc.dram_tensor("pbexp", [128, B * 1024], BF16, kind="ExternalInput")
    out_d = nc.dram_tensor("out", [TOK, DIM], F32, kind="ExternalOutput")

    with tile.TileContext(nc) as tc:
        with (
            tc.tile_pool(name="const", bufs=1) as cpool,
            tc.tile_pool(name="work", bufs=1) as wpool,
            tc.tile_pool(name="psP", bufs=1, space="PSUM") as psP,
        ):
            wq_sb = cpool.tile([128, 4, DIM], BF16)
            nc.gpsimd.dma_start(wq_sb[:], wq_d.ap().rearrange("(kb p) c -> p kb c", p=128))
            wkv_sb = cpool.tile([128, 4, 2 * DIM], BF16)
            nc.gpsimd.dma_start(wkv_sb[:], wkv_d.ap().rearrange("(kb p) c -> p kb c", p=128))
            wo_sb = cpool.tile([128, 4, DIM], BF16)
            nc.gpsimd.dma_start(wo_sb[:], wo_d.ap().rearrange("(kb p) c -> p kb c", p=128))
            cos_sb = cpool.tile([128, TILE_T], BF16)
            nc.gpsimd.dma_start(cos_sb[:], cos_d.ap())
            sin_sb = cpool.tile([128, TILE_T], BF16)
            nc.gpsimd.dma_start(sin_sb[:], sin_d.ap())
            pb_sb = cpool.tile([128, B * 1024], BF16)
            nc.gpsimd.dma_start(pb_sb[:], pb_d.ap())

            swap_mask = [(i ^ 1) for i in range(32)]

            def rope(tag, ps_in, nb=8):
                # rope(t)[p] = t[p]*cos[p] + t[p^1]*sin_signed[p]
                #            = t[p]*cos[p] + shuffle(t * sin_pre)[p]
                # with sin_pre[p] = sin_signed[p^1] (host-precomputed in sin_sb)
                t1 = wpool.tile([128, nb, TILE_T], BF16, tag=f"tt1", name=f"t1{tag}")
                nc.vector.scalar_tensor_tensor(
                    t1[:], ps_in[:], 1.0,
                    cos_sb[:].rearrange("p t -> p () t").broadcast_to((128, nb, TILE_T)),
                    Mult, Mult)
                t2s = wpool.tile([128, nb, TILE_T], BF16, tag=f"tt2", name=f"t2s{tag}")
                nc.vector.scalar_tensor_tensor(
                    t2s[:], ps_in[:], 1.0,
                    sin_sb[:].rearrange("p t -> p () t").broadcast_to((128, nb, TILE_T)),
                    Mult, Mult)
                sw = wpool.tile([128, nb * TILE_T], BF16, tag="sw", name=f"sw{tag}")
                nc.vector.stream_shuffle(sw[:], t2s[:].rearrange("p a t -> p (a t)"),
                                         swap_mask)
                r = wpool.tile([128, nb, TILE_T], BF16, tag=f"r{tag}", name=f"r{tag}")
                nc.vector.tensor_add(r[:], t1[:],
                                     sw[:].rearrange("p (a t) -> p a t", a=nb))
                return r

            def dbg_out(flat2d, nparts=128):
                # debug: dump a [nparts, <=2048] 2D AP (any dtype) as f32 into out rows 0-511
                w = flat2d.shape[-1]
                dmp = wpool.tile([128, 2048], F32, tag="dmp", name="dmp")
                nc.vector.memset(dmp[:], 0.0)
                nc.vector.tensor_copy(dmp[:nparts, :w], flat2d)
                nc.scalar.dma_start(
                    out_d.ap()[0:512, :].rearrange("(a p) c -> p a c", p=128),
                    dmp[:].rearrange("p (a c) -> p a c", a=4))

            for rep in range(repeat):
              for tt in range(n_tiles):
                b = tt // (N_TILES // B)
                focus = mask_flags[b]

                xsb = wpool.tile([128, 4, TILE_T], BF16, tag="xsb")
                nc.sync.dma_start(
                    xsb[:].rearrange("p a t -> p (a t)"),
                    x_d.ap().rearrange("(t p) c -> t p c", p=128)[tt])

                if focus:
                    # focus-present: attention is identity -> out = (x@wv)@wo
                    # compute vT (hidden-dim on partitions) like qT
                    psV = psP.tile([128, 4 * DIM], F32, tag="ps", name=f"psVf{rep}_{tt}")
                    for db in range(4):
                        for kb in range(4):
                            nc.tensor.matmul(psV[:, ts(db, DIM)],
                                             wkv_sb[:, kb, DIM + db * 128: DIM + (db + 1) * 128],
                                             xsb[:, kb, :],
                                             start=(kb == 0), stop=(kb == 3))
                    vt = wpool.tile([128, 4, TILE_T], BF16, tag="yt", name=f"vtf{rep}_{tt}")
                    nc.vector.tensor_copy(vt[:].rearrange("p a t -> p (a t)"), psV[:])
                    psO = psP.tile([128, 4 * DIM], F32, tag="ps", name=f"psOf{rep}_{tt}")
                    for tb in range(4):
                        for hb in range(4):
                            nc.tensor.matmul(psO[:, ts(tb, DIM)],
                                             vt[:, hb, ts(tb, 128)], wo_sb[:, hb, :],
                                             start=(hb == 0), stop=(hb == 3))
                    osb = wpool.tile([128, 4, DIM], F32, tag="osb", name=f"osbf{rep}_{tt}")
                    nc.vector.tensor_copy(osb[:].rearrange("p a t -> p (a t)"), psO[:])
                    nc.scalar.dma_start(
                        out_d.ap().rearrange("(t tb p) c -> t p tb c", tb=4, p=128)[tt],
                        osb[:])
                    continue

                # ---- qT, kT merged: head-dim on partitions, q at a=0-3, k at a=4-7
                psQK = psP.tile([128, 8, TILE_T], F32, tag="ps", name=f"psQK{rep}_{tt}")
                for db in range(4):
                    for kb in range(4):
                        nc.tensor.matmul(psQK[:, db, :],
                                         wq_sb[:, kb, ts(db, 128)], xsb[:, kb, :],
                                         start=(kb == 0), stop=(kb == 3))
                for db in range(4):
                    for kb in range(4):
                        nc.tensor.matmul(psQK[:, 4 + db, :],
                                         wkv_sb[:, kb, ts(db, 128)], xsb[:, kb, :],
                                         start=(kb == 0), stop=(kb == 3))
                if stage == "q":
                    dbg_out(psQK[:, :4].rearrange("p a t -> p (a t)")); continue

                rqk = rope("qk", psQK[:])
                if stage == "rq":
                    dbg_out(rqk[:, :4].rearrange("p a t -> p (a t)")); continue

                # repack to base-0 per-head layout [64, slot, t]:
                # slot h = q head h, slot 8+h = k head h
                rqk64 = wpool.tile([64, 16, TILE_T], BF16, tag="rqk64")
                nc.sync.dma_start(rqk64[:, 0::2], rqk[0:64])
                nc.sync.dma_start(rqk64[:, 1::2], rqk[64:128])

                # ---- v: natural layout (token on partitions) ----
                psV = psP.tile([128, 4 * DIM], F32, tag="ps", name=f"psV{rep}_{tt}")
                for tb in range(4):
                    for kb in range(4):
                        nc.tensor.matmul(psV[:, ts(tb, DIM)],
                                         xsb[:, kb, ts(tb, 128)],
                                         wkv_sb[:, kb, DIM:],
                                         start=(kb == 0), stop=(kb == 3))
                vsb = wpool.tile([128, 4, DIM], BF16, tag="vsb")
                nc.vector.tensor_copy(vsb[:].rearrange("p a t -> p (a t)"), psV[:])
                if stage == "v":
                    dbg_out(vsb[:].rearrange("p a t -> p (a t)")); continue

                # ---- sim: S^T per (tb, h) -> [128 j, (tb, h, i)] ----
                psS = psP.tile([128, 4096], F32, tag="ps", name=f"psS{rep}_{tt}")
                for tb in range(4):
                    for h in range(HEADS):
                        seg = tb * 1024 + h * 128
                        nc.tensor.matmul(
                            psS[:, seg:seg + 128],
                            rqk64[:, 8 + h, ts(tb, 128)],
                            rqk64[:, h, ts(tb, 128)],
                            start=(h % 4 == 0), stop=(h % 4 == 3),
                            skip_group_check=True)

                if stage == "sim":
                    dbg_out(psS[:, :2048]); continue
                esb = wpool.tile([128, 4096], BF16, tag="esb")
                nc.scalar.activation(esb[:], psS[:], Exp)

                # mask + pos_bias via multiply with exp(pb) table (0 kills)
                em = wpool.tile([128, 4, 1024], BF16, tag="em")
                nc.vector.tensor_mul(
                    em[:], esb[:].rearrange("p (a f) -> p a f", a=4),
                    pb_sb[:, b * 1024:(b + 1) * 1024]
                    .rearrange("p f -> p () f").broadcast_to((128, 4, 1024)))

                if stage == "em":
                    dbg_out(em[:].rearrange("p a f -> p (a f)")[:, :2048]); continue
                # softmax sums over j (partitions) -> broadcast to all partitions
                sums = wpool.tile([128, 4096], F32, tag="sums")
                nc.gpsimd.partition_all_reduce(sums[:], em[:].rearrange("p a f -> p (a f)"),
                                               channels=128,
                                               reduce_op=bass_isa.ReduceOp.add)
                if stage == "sums":
                    dbg_out(sums[:, :2048]); continue
                rcp = wpool.tile([128, 4096], F32, tag="rcp")
                nc.vector.reciprocal(rcp[:], sums[:])
                psb_t = wpool.tile([128, 4096], BF16, tag="psb")
                nc.vector.tensor_mul(psb_t[:], em[:].rearrange("p a f -> p (a f)"), rcp[:])
                if stage == "p":
                    dbg_out(psb_t[:, :2048]); continue

                # ---- AV: Y^T per head at base partition 0: [64 d, (h, tb, i)] ----
                psYT = psP.tile([64, 4096], F32, tag="ps", name=f"psYT{rep}_{tt}")
                for h in range(HEADS):
                    for tb in range(4):
                        nc.tensor.matmul(
                            psYT[:, h * 512 + tb * 128: h * 512 + (tb + 1) * 128],
                            vsb[:, tb, ts(h, 64)],
                            psb_t[:, tb * 1024 + h * 128: tb * 1024 + (h + 1) * 128],
                            start=(tb == 0), stop=(tb == 3),
                            skip_group_check=True)
                yt64 = wpool.tile([64, 8, 4, 128], BF16, tag="yt64")
                nc.vector.tensor_copy(yt64[:].rearrange("p a b i -> p (a b i)"), psYT[:])
                if stage == "yt64":
                    dbg_out(yt64[:, :4].rearrange("p a b i -> p (a b i)"), nparts=64); continue
                # repack to [128 hidden, (hb, tb, i)] via 2 partition-moving DMAs
                yt = wpool.tile([128, 4, 4, 128], BF16, tag="yt")
                nc.sync.dma_start(yt[0:64], yt64[:, 0::2])
                nc.sync.dma_start(yt[64:128], yt64[:, 1::2])
                if stage == "yt":
                    dbg_out(yt[:].rearrange("p a b i -> p (a b i)")); continue

                # ---- out projection (natural layout) ----
                psO = psP.tile([128, 4 * DIM], F32, tag="ps", name=f"psO{rep}_{tt}")
                for tb in range(4):
                    for hb in range(4):
                        nc.tensor.matmul(psO[:, ts(tb, DIM)],
                                         yt[:, hb, tb, :], wo_sb[:, hb, :],
                                         start=(hb == 0), stop=(hb == 3))
                osb = wpool.tile([128, 4, DIM], F32, tag="osb")
                nc.vector.tensor_copy(osb[:].rearrange("p a t -> p (a t)"), psO[:])
                nc.scalar.dma_start(
                    out_d.ap().rearrange("(t tb p) c -> t p tb c", tb=4, p=128)[tt],
                    osb[:])

    nc.compile()
    return nc


def _host_tables(pos_bias, focus_present_mask, inv_freq):
    # cos/sin in head-dim-on-partitions layout: [128, TILE_T]
    invf = np.asarray(inv_freq, np.float32)          # (32,)
    dd = np.arange(64)
    freq_d = invf[dd // 2]                           # (64,) per within-head dim
    pos = np.arange(TILE_T, dtype=np.float32) % N    # token pos within group
    theta = freq_d[:, None] * pos[None, :]           # (64, T)
    cos = np.cos(theta)
    sin = np.sin(theta)
    # sin_pre[p] = sin_signed[p^1] = sign0(p^1)*sin(theta_p) ; sign0 = -1 even/+1 odd
    sign_pre = np.where(dd % 2 == 0, 1.0, -1.0).astype(np.float32)
    cos_t = np.tile(cos, (2, 1)).astype(ml_dtypes.bfloat16)          # (128, T)
    sin_t = np.tile(sin * sign_pre[:, None], (2, 1)).astype(ml_dtypes.bfloat16)

    # pbexp[j, b*1024 + h*128 + i]
    pb = np.exp(np.asarray(pos_bias, np.float32))    # (8, 32, 32) pb[h, i, j]
    mask = np.asarray(focus_present_mask)
    pbexp = np.zeros((128, B * 1024), np.float32)
    eye = np.eye(N, dtype=bool)
    for b in range(B):
        base = pb.copy()
        if mask[b]:
            base = base * eye[None, :, :]
        for g in range(4):
            rows = slice(32 * g, 32 * g + 32)
            for h in range(HEADS):
                cols = slice(b * 1024 + h * 128 + 32 * g, b * 1024 + h * 128 + 32 * g + 32)
                pbexp[rows, cols] = base[h].T        # [j, i]
    return cos_t, sin_t, pbexp.astype(ml_dtypes.bfloat16)


_NC_CACHE = {}
TRACE = False
REPEAT = 1
LAST_RESULT = None


def kernel(x, pos_bias, focus_present_mask, w_q, w_kv, w_out, inv_freq):
    x = np.asarray(x)
    mask = tuple(bool(v) for v in np.asarray(focus_present_mask))
    cos_t, sin_t, pb_t = _host_tables(pos_bias, focus_present_mask, inv_freq)

    wq_bf = (np.asarray(w_q, np.float32) * (DIM_HEAD ** -0.5)).astype(ml_dtypes.bfloat16)
    wkv_bf = np.asarray(w_kv, np.float32).astype(ml_dtypes.bfloat16)
    wo_bf = np.asarray(w_out, np.float32).astype(ml_dtypes.bfloat16)

    if (mask, REPEAT) not in _NC_CACHE:
        _NC_CACHE[(mask, REPEAT)] = build_nc(mask, repeat=REPEAT)
    nc = _NC_CACHE[(mask, REPEAT)]

    xs = x.reshape(B, N_CORES, HW_SHARD, N, DIM)
    in_maps = []
    for c in range(N_CORES):
        xc = np.ascontiguousarray(xs[:, c]).reshape(TOK, DIM).astype(ml_dtypes.bfloat16)
        # [tile, token-in-tile, kb, p] -> [tile, p, kb, token]
        xc = np.ascontiguousarray(
            xc.reshape(N_TILES, TILE_T, 4, 128).transpose(0, 3, 2, 1)
        ).reshape(N_TILES * 128, 4 * TILE_T)
        in_maps.append(dict(
            x=xc, wq=wq_bf, wkv=wkv_bf, wo=wo_bf,
            cosb=cos_t, sinb=sin_t, pbexp=pb_t,
        ))

    global LAST_RESULT
    res = run_bass_kernel_spmd(nc, in_maps, core_ids=list(range(N_CORES)), trace=TRACE)
    LAST_RESULT = res
    outs = [r["out"].reshape(B, HW_SHARD, N, DIM) for r in res.results]
    return np.concatenate(outs, axis=1).astype(np.float32)

